# revision 5
# baseline (speedup 1.0000x reference)
"""FDTD simulator kernel for Trainium2 (Bass/Tile).

Grid 300x300, 300 time steps, batch 2. Each batch sample runs a full
independent FDTD solve on its own NeuronCore (embarrassingly parallel over
batch). Layout: grid rows (x) in partitions, packed as 3 chunks of <=128 rows
side by side in the free dim, each chunk padded with one guard column on both
sides (block width 302). Partition-direction shifts (x-derivatives) are done
with SBUF->SBUF DMA descriptors (which, unlike compute engines, may address
arbitrary partition ranges); y-derivatives are free-dim AP offsets against
maintained guard columns. Damping is applied as border strips (ACT row strips
with per-partition scale + DVE column strips). The soft source is staged by
DMA into a zeroed [128,200] tile (only row 30 written) and added with one DVE
op; the detector row is DMA'd to DRAM each step and intensities are computed
on the host.
"""
import json
import sys

import numpy as np

sys.path.insert(0, "/opt/trn_rl_repo")

# ---------------------------------------------------------------- constants
Nx = Ny = 300
PML_W = 10
CIRCLE_COUNT = 10
CENTER_SIZE = 200
CENTER_START = 50
PORTS = 4
PORT_WIDTH = 20
SOURCE_LOC = 30
DETECTOR_LOC = 270
SIM_STEPS = 300
DX = 25e-9
C0 = 299792458.0
WAVELENGTH = 1550e-9
COURANT = np.float32(0.99 / np.sqrt(2.0))
PERIOD = WAVELENGTH / C0
DT = float(COURANT) * DX / C0
OMEGA_T = np.float32(2.0 * np.pi * DT / PERIOD)
SIGMA_MAX = np.float32(0.5)
PORT_CENTERS = (60, 120, 180, 240)

BW = 302          # block width per chunk (guardL + 300 + guardR)
W = 3 * BW        # 906 packed free width
P = 128
SRC_Y0, SRC_Y1 = 50, 250      # union of port windows (cols 50..249)
SRC_W = SRC_Y1 - SRC_Y0


def _port_mask():
    m = np.zeros((PORTS, Ny), np.float32)
    for i, c in enumerate(PORT_CENTERS):
        m[i, c - PORT_WIDTH // 2:c + PORT_WIDTH // 2] = 1.0
    return m


def _damp_1d():
    s = np.zeros(Nx, np.float32)
    ramp = (np.arange(PML_W, 0, -1, dtype=np.float32) / PML_W) ** 3
    s[:PML_W] = SIGMA_MAX * ramp
    s[-PML_W:] = SIGMA_MAX * ramp[::-1]
    return np.exp(-s).astype(np.float32)


def _build_permittivity(radius):
    eps = np.ones((Nx, Ny), np.float32)
    for c in PORT_CENTERS:
        y0 = c - PORT_WIDTH // 2
        y1 = y0 + PORT_WIDTH
        eps[0:SOURCE_LOC, y0:y1] = 2.8
        eps[DETECTOR_LOC:, y0:y1] = 2.8
    xs = np.arange(CENTER_SIZE, dtype=np.float32)
    x, y = np.meshgrid(xs, xs, indexing="ij")
    spacing = CENTER_SIZE // CIRCLE_COUNT
    c1 = np.arange(CIRCLE_COUNT, dtype=np.float32) * spacing + spacing / 2
    xc = np.repeat(c1, CIRCLE_COUNT)
    yc = np.tile(c1, CIRCLE_COUNT)
    d2 = (x[None] - xc[:, None, None]) ** 2 + (y[None] - yc[:, None, None]) ** 2
    inside = np.any(d2 <= radius[:, None, None] ** 2, axis=0)
    core = np.where(inside, np.float32(0.0), np.float32(1.0)) * np.float32(1.8) + 1.0
    eps[CENTER_START:CENTER_START + CENTER_SIZE,
        CENTER_START:CENTER_START + CENTER_SIZE] = core
    return eps


def _pack(full):
    """(300,300) -> [128,906] data cols, guards/pad zero."""
    t = np.zeros((P, W), np.float32)
    for c in range(3):
        rows = full[128 * c:128 * (c + 1)]
        t[:rows.shape[0], BW * c + 1:BW * c + 301] = rows
    return t


# ------------------------------------------------- walrus multi-wait fixup
def _split_multiwait_json(bj: bytes, cap: int = 1) -> bytes:
    j = json.loads(bj)
    ctr = 0
    changed = False
    for fn in j.get("functions", []):
        for bb in fn.get("blocks", []):
            new = []
            for inst in bb.get("instructions", []):
                si = inst.get("sync_info")
                ow = (si or {}).get("on_wait") or []
                if len(ow) > cap:
                    changed = True
                    extra, keep = ow[:-cap], ow[-cap:]
                    for i in range(0, len(extra), cap):
                        ctr += 1
                        new.append({
                            "debug": inst.get("debug", 0),
                            "engine": inst["engine"],
                            "ins": [], "outs": [],
                            "name": f"I-mwfix{ctr}",
                            "opcode": "NoOp",
                            "sync_info": {"on_update": [],
                                          "on_wait": extra[i:i + cap]},
                        })
                    si["on_wait"] = keep
                new.append(inst)
            bb["instructions"] = new
    return json.dumps(j).encode() if changed else bj


def _install_mwfix():
    from concourse import bass_utils, bass2jax
    if getattr(bass_utils.compile_bir_kernel, "_mwfix", False):
        return
    orig = bass_utils.compile_bir_kernel

    def patched(bir_json, tmpdir, neff_name="file.neff"):
        return orig(_split_multiwait_json(bir_json), tmpdir, neff_name=neff_name)

    patched._mwfix = True
    bass_utils.compile_bir_kernel = patched
    bass2jax.compile_bir_kernel = patched


# ---------------------------------------------------------------- bass build
def build_nc(n_steps: int, debug_fields: bool = False):
    import concourse.bass as bass
    import concourse.mybir as mybir
    from concourse.tile import TileContext

    f32 = mybir.dt.float32
    AO = mybir.AluOpType
    AF = mybir.ActivationFunctionType

    nc = bass.Bass("TRN2", target_bir_lowering=False, debug=False)
    ieps_d = nc.dram_tensor("ieps", [P, W], f32, kind="ExternalInput").ap()
    damp_d = nc.dram_tensor("damp", [P, W], f32, kind="ExternalInput").ap()
    src_d = nc.dram_tensor("src", [SIM_STEPS, SRC_W], f32, kind="ExternalInput").ap()
    det_d = nc.dram_tensor("det", [SIM_STEPS, Ny], f32, kind="ExternalOutput").ap()
    dbg = {}
    if debug_fields:
        for nm in ("ezout", "hxout", "hyout"):
            dbg[nm] = nc.dram_tensor(nm, [P, W], f32, kind="ExternalOutput").ap()

    with TileContext(nc) as tc:
        with tc.tile_pool(name="sb", bufs=1) as pool:
            Ez = pool.tile([P, W], f32, tag="Ez")
            Hx = pool.tile([P, W], f32, tag="Hx")
            Hy = pool.tile([P, W], f32, tag="Hy")
            EzS = pool.tile([P, W], f32, tag="EzS")
            HyM = pool.tile([P, W], f32, tag="HyM")
            D1 = pool.tile([P, W], f32, tag="D1")
            D2 = pool.tile([P, W], f32, tag="D2")
            D3 = pool.tile([P, W], f32, tag="D3")
            D4 = pool.tile([P, W], f32, tag="D4")
            Wt = pool.tile([P, W], f32, tag="Wt")
            M = pool.tile([P, W], f32, tag="M")
            ieps = pool.tile([P, W], f32, tag="ieps")
            damp = pool.tile([P, W], f32, tag="damp")
            slots = []
            for i in range(4):
                slot_t = pool.tile([P, SRC_W], f32, tag=f"slot{i}", name=f"slot{i}")
                slots.append(slot_t)

            # ---- init
            for t_ in (Ez, Hx, Hy, EzS, HyM, D1, D2, D3, D4, Wt, M):
                nc.vector.memset(t_[:], 0.0)
            for s in slots:
                nc.vector.memset(s[:], 0.0)
            nc.sync.dma_start(ieps[:], ieps_d[:])
            nc.sync.dma_start(damp[:], damp_d[:])

            # strided views for guard cols and dy strips
            def blocks(tile_):
                return tile_.rearrange("p (c w) -> p c w", c=3)

            for t in range(n_steps):
                slot = slots[t % 4]
                # stage source row for this step (only row 30 nonzero)
                nc.sync.dma_start(slot[30:31, :], src_d[t:t + 1, :])

                # ---- Ez partition shift (Ez_s[p] = Ez[x+1]); row-299 pad and
                # chunk seams fixed by two overwriting descriptors (ordered)
                nc.sync.dma_start(EzS[0:127, :], Ez[1:128, :])
                nc.sync.dma_start(EzS[43:44, 604:906], Ez[43:44, 604:906])
                nc.sync.dma_start(EzS[127:128, 0:604], Ez[0:1, 302:906])

                # ---- H phase
                nc.vector.tensor_tensor(D1[:, 0:905], Ez[:, 1:906], Ez[:, 0:905], AO.subtract)
                nc.vector.tensor_tensor(Hx[:], Hx[:], D1[:], AO.subtract)
                nc.vector.tensor_tensor(Hx[:], Hx[:], damp[:], AO.mult)
                nc.vector.tensor_tensor(D2[:], EzS[:], Ez[:], AO.subtract)
                nc.vector.tensor_tensor(Hy[:], Hy[:], D2[:], AO.add)
                nc.vector.tensor_tensor(Hy[:], Hy[:], damp[:], AO.mult)
                # Hx guardL maintenance (cols 0,302,604 <- 1,303,605)
                bHx = blocks(Hx)
                nc.vector.tensor_copy(bHx[:, :, 0], bHx[:, :, 1])

                # ---- Hy partition shift (Hy_m[p] = Hy[x-1], x=0 row -> 0)
                nc.sync.dma_start(HyM[1:128, :], Hy[0:127, :])
                nc.sync.dma_start(HyM[0:1, 302:906], Hy[127:128, 0:604])

                # ---- E phase
                nc.vector.tensor_tensor(D3[:, 1:906], Hx[:, 1:906], Hx[:, 0:905], AO.subtract)
                nc.vector.tensor_tensor(D4[:], Hy[:], HyM[:], AO.subtract)
                nc.vector.tensor_tensor(Wt[:], D4[:], D3[:], AO.subtract)
                nc.vector.tensor_tensor(M[:], ieps[:], Wt[:], AO.mult)  # ieps = damp*C^2/eps
                nc.vector.tensor_tensor(Ez[:], Ez[:], damp[:], AO.mult)
                nc.vector.tensor_tensor(Ez[:], Ez[:], M[:], AO.add)
                # source add: Ez[:, cols 51..251) += slot (rows != 30 are 0)
                nc.vector.tensor_tensor(Ez[:, 1 + SRC_Y0:1 + SRC_Y1],
                                        Ez[:, 1 + SRC_Y0:1 + SRC_Y1],
                                        slot[:], AO.add)
                # Ez guardR maintenance (cols 301,603,905 <- 300,602,904)
                bEz = blocks(Ez)
                nc.vector.tensor_copy(bEz[:, :, 301], bEz[:, :, 300])
                # detector row -> DRAM (chunk2 partition 14, data cols)
                nc.sync.dma_start(det_d[t:t + 1, :], Ez[14:15, 605:905])
            if debug_fields:
                nc.sync.dma_start(dbg["ezout"][:], Ez[:])
                nc.sync.dma_start(dbg["hxout"][:], Hx[:])
                nc.sync.dma_start(dbg["hyout"][:], Hy[:])
    return nc


def _build_consts(radius_matrix):
    radius = np.where(radius_matrix < 0.3, 0.0, radius_matrix).reshape(-1).astype(np.float32) * 10.0
    eps = _build_permittivity(radius)
    ieps2 = (np.float32(COURANT) * np.float32(COURANT) / eps).astype(np.float32)
    d1 = _damp_1d()
    damp_full = np.outer(d1, d1).astype(np.float32)
    damp_packed = _pack(damp_full)
    dampieps_packed = _pack((damp_full * ieps2).astype(np.float32))
    return dampieps_packed, damp_packed


def _build_src(source_row_phases):
    """phases: (PORTS,) in radians. Returns [SIM_STEPS, SRC_W] f32."""
    pm = _port_mask()[:, SRC_Y0:SRC_Y1]  # (4,200)
    t = np.arange(SIM_STEPS, dtype=np.float32)[:, None]
    amp = np.sin(OMEGA_T * t + source_row_phases[None, :]).astype(np.float32)  # (T,4)
    return (amp @ pm).astype(np.float32)


_nc_cache = {}


def _get_nc(n_steps):
    if n_steps not in _nc_cache:
        _nc_cache[n_steps] = build_nc(n_steps)
    return _nc_cache[n_steps]


def kernel(source: np.ndarray, radius_matrix: np.ndarray) -> np.ndarray:
    _install_mwfix()
    from concourse.bass_utils import run_bass_kernel_spmd

    source = np.asarray(source, np.float32)
    radius_matrix = np.asarray(radius_matrix, np.float32)
    B = source.shape[0]

    dampieps_packed, damp_packed = _build_consts(radius_matrix)
    phases = source * np.float32(np.pi)

    in_maps = []
    for b in range(B):
        in_maps.append({
            "ieps": dampieps_packed, "damp": damp_packed,
            "src": _build_src(phases[b]),
        })
    nc = _get_nc(SIM_STEPS)
    r = run_bass_kernel_spmd(nc, in_maps, core_ids=list(range(B)))

    pm = _port_mask()  # (4,300)
    out = np.zeros((B, SIM_STEPS, PORTS), np.float32)
    for b in range(B):
        det = r.results[b]["det"]  # (300,300)
        out[b] = (det ** 2) @ pm.T / np.float32(PORT_WIDTH)
    return out
